# revision 3
# baseline (speedup 1.0000x reference)
"""KNN graph kernel (DenseDilatedKnnGraph) for Trainium2, 8 NeuronCores.

Problem: x [2, 192, 8192, 1] fp32 -> edge_index [2, 2, 8192, 9] int32.
reference: L2-normalize x along C, pairwise sq-dists over N, top-9 (k=9,
dilation=1) nearest neighbors (indices), stacked with center indices.

Math used here: for normalized points, ranking by -dist == ranking by
cosine = Xn^T Xn. The nearest neighbor is always the point itself
(cos=1 >> all others for this data), so the device computes the top-8
of the Gram matrix with the self-column masked out; the host prepends
the self index.

Sharding: 8 cores = 2 batches x 4 query-row-blocks of 2048. Each core
gets the full batch slice with its columns ROTATED so its own query
block sits at columns 0..2047 (keeps the SPMD program identical across
cores: the self-match diagonal is at a static position). Host maps
returned neighbor indices back by adding the rotation offset mod N.

Per core device pipeline:
  1. DMA x [192, 8192] into SBUF (two K-tiles, 64 zero-padded rows).
  2. norms^2 via ones-matmul over partitions; sqrt (ACT); reciprocal in
     a [128, 64] transposed layout (DVE, via DRAM bounce); broadcast
     1/norm over partitions with a K=1 ones matmul; normalize in place.
  3. For each of 16 query row-tiles [128 x 8192]: Gram via fp32 matmuls
     (PSUM, 512-col chunks), evacuate to SBUF (ACT), add -20 on the
     self diagonal, DVE max (top-8 values) + max_index (their column
     indices, jax top_k tie semantics), DMA indices out as uint16.
"""

import numpy as np

B = 2
C = 192
N = 8192
NCORES = 8
RBLK = N // 4  # 2048 query rows per core
CHUNK = 512
NCHUNK = N // CHUNK  # 16
NT = RBLK // 128  # 16 row tiles per core
NEG = -20.0

_cache = {}


def _build_nc(nt=NT):
    import concourse.bacc as bacc
    import concourse.mybir as mybir
    from concourse.bass import ts
    from concourse.tile import TileContext

    f32 = mybir.dt.float32
    u16 = mybir.dt.uint16

    nc = bacc.Bacc("TRN2")

    xin = nc.dram_tensor("xin", [C, N], f32, kind="ExternalInput")
    idx_out = nc.dram_tensor("idx8", [RBLK, 8], u16, kind="ExternalOutput")
    nrm_dram = nc.dram_tensor("nrm_scratch", [N], f32, kind="Internal")
    rn_dram = nc.dram_tensor("rn_scratch", [N], f32, kind="Internal")

    onesk_d = nc.inline_tensor(np.ones((128, 1), np.float32), name="onesk")
    onesm_d = nc.inline_tensor(np.ones((1, 128), np.float32), name="onesm")
    eye_d = nc.inline_tensor(np.eye(128, dtype=np.float32) * NEG, name="eyeneg")

    with TileContext(nc) as tc:
        with (
            tc.tile_pool(name="consts", bufs=1) as cpool,
            tc.tile_pool(name="xpool", bufs=1) as xpool,
            tc.tile_pool(name="spool", bufs=3) as spool,
            tc.tile_pool(name="gpool", bufs=2) as gpool,
            tc.tile_pool(name="vpool", bufs=3) as vpool,
            tc.tile_pool(name="npsum", bufs=2, space="PSUM") as npsum,
            tc.tile_pool(name="bpsum", bufs=2, space="PSUM") as bpsum,
            tc.tile_pool(name="gpsum", bufs=4, space="PSUM") as gpsum,
        ):
            ck = cpool.tile([128, 1], f32)
            nc.sync.dma_start(ck, onesk_d[:, :])
            cm = cpool.tile([1, 128], f32)
            nc.sync.dma_start(cm, onesm_d[:, :])
            eye = cpool.tile([128, 128], f32)
            nc.sync.dma_start(eye, eye_d[:, :])

            # x in [C, N] layout: channels 0..127 in xA, 128..191 in xB
            # (rows 64..127 of xB zeroed so K=128 matmuls see zero padding).
            xA = xpool.tile([128, N], f32)
            nc.sync.dma_start(xA, xin[0:128, :])
            xB = xpool.tile([128, N], f32)
            nc.vector.memset(xB[64:128, :], 0.0)
            nc.sync.dma_start(xB[0:64, :], xin[128:192, :])

            # ---- norms ----
            nrm = cpool.tile([1, N], f32)
            for cc in range(NCHUNK):
                sl = ts(cc, CHUNK)
                sqA = spool.tile([128, CHUNK], f32)
                nc.scalar.square(sqA, xA[:, sl])
                sqB = spool.tile([128, CHUNK], f32)
                nc.scalar.square(sqB, xB[:, sl])
                nps = npsum.tile([1, CHUNK], f32)
                nc.tensor.matmul(nps, ck, sqA, start=True, stop=False)
                nc.tensor.matmul(nps, ck, sqB, start=False, stop=True)
                nc.scalar.sqrt(nrm[:, sl], nps)

            # reciprocal in [128, 64] layout (DVE divide is per-lane; a
            # [1, N] reciprocal would run on one lane)
            nc.sync.dma_start(nrm_dram[None, :], nrm)
            nrmT = cpool.tile([128, N // 128], f32)
            nc.sync.dma_start(nrmT, nrm_dram[:].rearrange("(p f) -> p f", p=128))
            rnT = cpool.tile([128, N // 128], f32)
            nc.vector.reciprocal(rnT, nrmT)
            nc.sync.dma_start(rn_dram[:].rearrange("(p f) -> p f", p=128), rnT)
            rn = cpool.tile([1, N], f32)
            nc.sync.dma_start(rn, rn_dram[None, :])

            # ---- normalize x in place: x *= (1/norm) broadcast over C ----
            for cc in range(NCHUNK):
                sl = ts(cc, CHUNK)
                bps = bpsum.tile([128, CHUNK], f32)
                nc.tensor.matmul(bps, cm, rn[:, sl], start=True, stop=True)
                nc.vector.tensor_mul(xA[:, sl], xA[:, sl], bps)
                nc.vector.tensor_mul(xB[0:64, sl], xB[0:64, sl], bps[0:64, :])

            # ---- Gram + top-8 per 128-row query tile ----
            for t in range(nt):
                tsl = ts(t, 128)
                g = gpool.tile([128, N], f32)
                for cc in range(NCHUNK):
                    sl = ts(cc, CHUNK)
                    ps = gpsum.tile([128, CHUNK], f32)
                    nc.tensor.matmul(ps, xA[:, tsl], xA[:, sl], start=True, stop=False)
                    nc.tensor.matmul(ps, xB[:, tsl], xB[:, sl], start=False, stop=True)
                    nc.scalar.copy(g[:, sl], ps)
                # knock out the self-match diagonal (query p == column 128t+p)
                nc.vector.tensor_add(g[:, tsl], g[:, tsl], eye)
                v8 = vpool.tile([128, 8], f32)
                nc.vector.max(out=v8, in_=g)
                i8 = vpool.tile([128, 8], u16)
                nc.vector.max_index(i8, v8, g)
                nc.sync.dma_start(idx_out[tsl, :], i8)

    nc.compile()
    return nc


def _get_nc():
    if "nc" not in _cache:
        _cache["nc"] = _build_nc()
    return _cache["nc"]


def shard_inputs(x):
    """x: [B, C, N, 1] -> list of 8 per-core input maps (rotated columns)."""
    xs = np.ascontiguousarray(np.asarray(x, dtype=np.float32).reshape(B, C, N))
    in_maps = []
    for c in range(NCORES):
        b, r = divmod(c, 4)
        s = r * RBLK
        xb = xs[b]
        rot = np.ascontiguousarray(np.roll(xb, -s, axis=1)) if s else xb
        in_maps.append({"xin": rot})
    return in_maps


def assemble(results):
    """results: list of 8 dicts with 'idx8' [RBLK, 8] uint16 -> edge_index."""
    nn = np.empty((B, N, 9), np.int32)
    for c in range(NCORES):
        b, r = divmod(c, 4)
        s = r * RBLK
        i8 = results[c]["idx8"].astype(np.int64)
        nn[b, s : s + RBLK, 1:9] = (i8 + s) % N
        nn[b, s : s + RBLK, 0] = np.arange(s, s + RBLK)
    center = np.broadcast_to(np.arange(N, dtype=np.int32)[None, :, None], (B, N, 9))
    return np.ascontiguousarray(np.stack([nn, center], axis=0).astype(np.int32))


def kernel(x, _trace=False, **trace_kwargs):
    from concourse.bass_utils import run_bass_kernel_spmd

    nc = _get_nc()
    in_maps = shard_inputs(x)
    res = run_bass_kernel_spmd(
        nc, in_maps, core_ids=list(range(NCORES)), trace=_trace, **trace_kwargs
    )
    _cache["last_results"] = res
    return assemble(res.results)


# revision 5
# speedup vs baseline: 1.1012x; 1.1012x over previous
"""KNN graph kernel (DenseDilatedKnnGraph) for Trainium2, 8 NeuronCores.

Problem: x [2, 192, 8192, 1] fp32 -> edge_index [2, 2, 8192, 9] int32.
reference: L2-normalize x along C, pairwise sq-dists over N, top-9 (k=9,
dilation=1) nearest neighbors (indices), stacked with center indices.

Math used here: for normalized points, ranking by -dist == ranking by
cosine = Xn^T Xn. The nearest neighbor is always the point itself
(cos=1 >> all others for this data), so the device computes the top-8
of the Gram matrix with the self-column masked out; the host prepends
the self index.

Sharding: 8 cores = 2 batches x 4 query-row-blocks of 2048. Each core
gets the full batch slice with its columns ROTATED so its own query
block sits at columns 0..2047 (keeps the SPMD program identical across
cores: the self-match diagonal is at a static position). Host maps
returned neighbor indices back by adding the rotation offset mod N.

Per core device pipeline:
  1. DMA x [192, 8192] into SBUF (two K-tiles, 64 zero-padded rows).
  2. norms^2 via ones-matmul over partitions; sqrt (ACT); reciprocal in
     a [128, 64] transposed layout (DVE, via DRAM bounce); broadcast
     1/norm over partitions with a K=1 ones matmul; normalize in place.
  3. For each of 16 query row-tiles [128 x 8192]: Gram via fp32 matmuls
     (PSUM, 512-col chunks), evacuate to SBUF (ACT), add -20 on the
     self diagonal, DVE max (top-8 values) + max_index (their column
     indices, jax top_k tie semantics), DMA indices out as uint16.
"""

import numpy as np

B = 2
C = 192
N = 8192
NCORES = 8
RBLK = N // 4  # 2048 query rows per core
CHUNK = 512
NCHUNK = N // CHUNK  # 16
NT = RBLK // 128  # 16 row tiles per core
NEG = -20.0

_cache = {}


def _build_nc(nt=NT):
    import concourse.bacc as bacc
    import concourse.mybir as mybir
    from concourse.bass import ts
    from concourse.tile import TileContext

    f32 = mybir.dt.float32
    u16 = mybir.dt.uint16

    nc = bacc.Bacc("TRN2")

    xin = nc.dram_tensor("xin", [C, N], f32, kind="ExternalInput")
    idx_out = nc.dram_tensor("idx8", [RBLK, 8], u16, kind="ExternalOutput")
    nrm_dram = nc.dram_tensor("nrm_scratch", [N], f32, kind="Internal")
    rn_dram = nc.dram_tensor("rn_scratch", [N], f32, kind="Internal")

    onesk_d = nc.inline_tensor(np.ones((128, 1), np.float32), name="onesk")
    eye_d = nc.inline_tensor(np.eye(128, dtype=np.float32) * NEG, name="eyeneg")

    DCH = 2048  # input DMA chunk

    with TileContext(nc) as tc:
        with (
            tc.tile_pool(name="consts", bufs=1) as cpool,
            tc.tile_pool(name="xpool", bufs=1) as xpool,
            tc.tile_pool(name="spool", bufs=3) as spool,
            tc.tile_pool(name="rpool", bufs=3) as rpool,
            tc.tile_pool(name="gpool", bufs=2) as gpool,
            tc.tile_pool(name="vpool", bufs=3) as vpool,
            tc.tile_pool(name="npsum", bufs=2, space="PSUM") as npsum,
            tc.tile_pool(name="gpsum", bufs=6, space="PSUM") as gpsum,
        ):
            ck = cpool.tile([128, 1], f32)
            nc.sync.dma_start(ck, onesk_d[:, :])
            eye = cpool.tile([128, 128], f32)
            nc.sync.dma_start(eye, eye_d[:, :])

            # x in [C, N] layout: channels 0..127 in xA, 128..191 in xB
            # (rows 64..127 of xB zeroed so K=128 matmuls see zero padding).
            xA = xpool.tile([128, N], f32)
            xB = xpool.tile([128, N], f32)
            nc.gpsimd.memset(xB[64:128, :], 0.0)
            for dc in range(N // DCH):
                dsl = ts(dc, DCH)
                nc.sync.dma_start(xA[:, dsl], xin[0:128, dsl])
                nc.sync.dma_start(xB[0:64, dsl], xin[128:192, dsl])

            # ---- norms ----
            nrm = cpool.tile([1, N], f32)
            for cc in range(NCHUNK):
                sl = ts(cc, CHUNK)
                sqA = spool.tile([128, CHUNK], f32)
                nc.scalar.square(sqA, xA[:, sl])
                sqB = spool.tile([128, CHUNK], f32)
                nc.scalar.square(sqB, xB[:, sl])
                nps = npsum.tile([1, CHUNK], f32)
                nc.tensor.matmul(nps, ck, sqA, start=True, stop=False)
                nc.tensor.matmul(nps, ck, sqB, start=False, stop=True)
                nc.scalar.sqrt(nrm[:, sl], nps)

            # reciprocal in [128, 64] layout (DVE divide is per-lane; a
            # [1, N] reciprocal would run on one lane)
            nc.sync.dma_start(nrm_dram[None, :], nrm)
            nrmT = cpool.tile([128, N // 128], f32)
            nc.sync.dma_start(nrmT, nrm_dram[:].rearrange("(p f) -> p f", p=128))
            rnT = cpool.tile([128, N // 128], f32)
            nc.vector.reciprocal(rnT, nrmT)
            nc.sync.dma_start(rn_dram[:].rearrange("(p f) -> p f", p=128), rnT)

            # ---- normalize x in place: x *= (1/norm) broadcast over C ----
            # 1/norm row is broadcast across partitions by DMA (step-0 AP);
            # A-half multiplies on DVE, B-half on GpSimd (runs in parallel).
            for cc in range(NCHUNK):
                sl = ts(cc, CHUNK)
                rnb = rpool.tile([128, CHUNK], f32)
                nc.sync.dma_start(
                    rnb, rn_dram[None, ts(cc, CHUNK)].to_broadcast([128, CHUNK])
                )
                nc.vector.tensor_mul(xA[:, sl], xA[:, sl], rnb)
                nc.gpsimd.tensor_mul(xB[0:64, sl], xB[0:64, sl], rnb[0:64, :])

            # ---- Gram + top-8 per 128-row query tile ----
            for t in range(nt):
                tsl = ts(t, 128)
                g = gpool.tile([128, N], f32)
                for cc in range(NCHUNK):
                    sl = ts(cc, CHUNK)
                    ps = gpsum.tile([128, CHUNK], f32)
                    nc.tensor.matmul(ps, xA[:, tsl], xA[:, sl], start=True, stop=False)
                    nc.tensor.matmul(ps, xB[:, tsl], xB[:, sl], start=False, stop=True)
                    nc.scalar.copy(g[:, sl], ps)
                # knock out the self-match diagonal (query p == column 128t+p)
                nc.vector.tensor_add(g[:, tsl], g[:, tsl], eye)
                v8 = vpool.tile([128, 8], f32)
                nc.vector.max(out=v8, in_=g)
                i8 = vpool.tile([128, 8], u16)
                nc.vector.max_index(i8, v8, g)
                nc.sync.dma_start(idx_out[tsl, :], i8)

    nc.compile()
    return nc


def _get_nc():
    if "nc" not in _cache:
        _cache["nc"] = _build_nc()
    return _cache["nc"]


def shard_inputs(x):
    """x: [B, C, N, 1] -> list of 8 per-core input maps (rotated columns)."""
    xs = np.ascontiguousarray(np.asarray(x, dtype=np.float32).reshape(B, C, N))
    in_maps = []
    for c in range(NCORES):
        b, r = divmod(c, 4)
        s = r * RBLK
        xb = xs[b]
        rot = np.ascontiguousarray(np.roll(xb, -s, axis=1)) if s else xb
        in_maps.append({"xin": rot})
    return in_maps


def assemble(results):
    """results: list of 8 dicts with 'idx8' [RBLK, 8] uint16 -> edge_index."""
    nn = np.empty((B, N, 9), np.int32)
    for c in range(NCORES):
        b, r = divmod(c, 4)
        s = r * RBLK
        i8 = results[c]["idx8"].astype(np.int64)
        nn[b, s : s + RBLK, 1:9] = (i8 + s) % N
        nn[b, s : s + RBLK, 0] = np.arange(s, s + RBLK)
    center = np.broadcast_to(np.arange(N, dtype=np.int32)[None, :, None], (B, N, 9))
    return np.ascontiguousarray(np.stack([nn, center], axis=0).astype(np.int32))


def kernel(x, _trace=False, **trace_kwargs):
    from concourse.bass_utils import run_bass_kernel_spmd

    nc = _get_nc()
    in_maps = shard_inputs(x)
    res = run_bass_kernel_spmd(
        nc, in_maps, core_ids=list(range(NCORES)), trace=_trace, **trace_kwargs
    )
    _cache["last_results"] = res
    return assemble(res.results)


# revision 7
# speedup vs baseline: 1.2209x; 1.1087x over previous
"""KNN graph kernel (DenseDilatedKnnGraph) for Trainium2, 8 NeuronCores.

Problem: x [2, 192, 8192, 1] fp32 -> edge_index [2, 2, 8192, 9] int32.
reference: L2-normalize x along C, pairwise sq-dists over N, top-9 (k=9,
dilation=1) nearest neighbors (indices), stacked with center indices.

Math used here: for normalized points, ranking by -dist == ranking by
cosine = Xn^T Xn. The nearest neighbor is always the point itself
(cos=1 >> all others for this data), so the device computes the top-8
of the Gram matrix with the self-column masked out; the host prepends
the self index.

Sharding: 8 cores = 2 batches x 4 query-row-blocks of 2048. Each core
gets the full batch slice with its columns ROTATED so its own query
block sits at columns 0..2047 (keeps the SPMD program identical across
cores: the self-match diagonal is at a static position). Host maps
returned neighbor indices back by adding the rotation offset mod N.

Per core device pipeline:
  1. DMA x [192, 8192] into SBUF (two K-tiles, 64 zero-padded rows).
  2. norms^2 via ones-matmul over partitions; sqrt (ACT); reciprocal in
     a [128, 64] transposed layout (DVE, via DRAM bounce); broadcast
     1/norm over partitions with a K=1 ones matmul; normalize in place.
  3. For each of 16 query row-tiles [128 x 8192]: Gram via fp32 matmuls
     (PSUM, 512-col chunks), evacuate to SBUF (ACT), add -20 on the
     self diagonal, DVE max (top-8 values) + max_index (their column
     indices, jax top_k tie semantics), DMA indices out as uint16.
"""

import numpy as np

B = 2
C = 192
N = 8192
NCORES = 8
RBLK = N // 4  # 2048 query rows per core
CHUNK = 512
NCHUNK = N // CHUNK  # 16
NT = RBLK // 128  # 16 row tiles per core
NEG = -20.0

_cache = {}

# "fp32": plain fp32 Gram (LOW_HIGH, 4 HW passes per chunk pair)
# "fp16x3": h/l fp16 split, 6 single-cycle passes (h.h + h.l + l.h), ~1e-8
#           systematic error (PE computes fp16 subnormals exactly; verified)
MODE = "fp16x3"


def _build_nc(nt=NT, mode=None):
    import concourse.bacc as bacc
    import concourse.mybir as mybir
    from concourse.bass import ts
    from concourse.tile import TileContext

    if mode is None:
        mode = MODE
    f32 = mybir.dt.float32
    f16 = mybir.dt.float16
    u16 = mybir.dt.uint16

    nc = bacc.Bacc("TRN2")

    xin = nc.dram_tensor("xin", [C, N], f32, kind="ExternalInput")
    idx_out = nc.dram_tensor("idx8", [RBLK, 8], u16, kind="ExternalOutput")
    nrm_dram = nc.dram_tensor("nrm_scratch", [N], f32, kind="Internal")
    rn_dram = nc.dram_tensor("rn_scratch", [N], f32, kind="Internal")

    onesk_d = nc.inline_tensor(np.ones((128, 1), np.float32), name="onesk")
    eye_d = nc.inline_tensor(np.eye(128, dtype=np.float32) * NEG, name="eyeneg")

    DCH = 2048  # input DMA chunk

    with TileContext(nc) as tc:
        with (
            tc.tile_pool(name="consts", bufs=1) as cpool,
            tc.tile_pool(name="xpool", bufs=1) as xpool,
            tc.tile_pool(name="spool", bufs=3) as spool,
            tc.tile_pool(name="rpool", bufs=3) as rpool,
            tc.tile_pool(name="gpool", bufs=2) as gpool,
            tc.tile_pool(name="vpool", bufs=3) as vpool,
            tc.tile_pool(name="npsum", bufs=2, space="PSUM") as npsum,
            tc.tile_pool(name="gpsum", bufs=6, space="PSUM") as gpsum,
        ):
            ck = cpool.tile([128, 1], f32)
            nc.sync.dma_start(ck, onesk_d[:, :])
            eye = cpool.tile([128, 128], f32)
            nc.sync.dma_start(eye, eye_d[:, :])

            if mode == "fp32":
                # x in [C, N] layout: channels 0..127 in xA, 128..191 in xB
                # (rows 64..127 of xB zeroed for K=128 zero-padded matmuls).
                xA = xpool.tile([128, N], f32)
                xB = xpool.tile([128, N], f32)
                nc.gpsimd.memset(xB[64:128, :], 0.0)
                for dc in range(N // DCH):
                    dsl = ts(dc, DCH)
                    nc.sync.dma_start(xA[:, dsl], xin[0:128, dsl])
                    nc.sync.dma_start(xB[0:64, dsl], xin[128:192, dsl])

                nrm = cpool.tile([1, N], f32)
                for cc in range(NCHUNK):
                    sl = ts(cc, CHUNK)
                    sqA = spool.tile([128, CHUNK], f32)
                    nc.scalar.square(sqA, xA[:, sl])
                    sqB = spool.tile([128, CHUNK], f32)
                    nc.scalar.square(sqB, xB[:, sl])
                    nps = npsum.tile([1, CHUNK], f32)
                    nc.tensor.matmul(nps, ck, sqA, start=True, stop=False)
                    nc.tensor.matmul(nps, ck, sqB, start=False, stop=True)
                    nc.scalar.sqrt(nrm[:, sl], nps)
                nc.sync.dma_start(nrm_dram[None, :], nrm)
            else:
                # streaming: square chunks in place, never keep fp32 x
                for cc in range(NCHUNK):
                    sl = ts(cc, CHUNK)
                    xa = spool.tile([128, CHUNK], f32, tag="xa")
                    nc.sync.dma_start(xa, xin[0:128, sl])
                    xb = spool.tile([128, CHUNK], f32, tag="xb")
                    nc.gpsimd.memset(xb[64:128, :], 0.0)
                    nc.sync.dma_start(xb[0:64, :], xin[128:192, sl])
                    nc.scalar.square(xa, xa)
                    nc.scalar.square(xb, xb)
                    nps = npsum.tile([1, CHUNK], f32)
                    nc.tensor.matmul(nps, ck, xa, start=True, stop=False)
                    nc.tensor.matmul(nps, ck, xb, start=False, stop=True)
                    nrmc = spool.tile([1, CHUNK], f32, tag="nrmc")
                    nc.scalar.sqrt(nrmc, nps)
                    nc.sync.dma_start(nrm_dram[None, ts(cc, CHUNK)], nrmc)

            # reciprocal in [128, 64] layout (DVE divide is per-lane; a
            # [1, N] reciprocal would run on one lane)
            nrmT = cpool.tile([128, N // 128], f32)
            nc.sync.dma_start(nrmT, nrm_dram[:].rearrange("(p f) -> p f", p=128))
            rnT = cpool.tile([128, N // 128], f32)
            nc.vector.reciprocal(rnT, nrmT)
            nc.sync.dma_start(rn_dram[:].rearrange("(p f) -> p f", p=128), rnT)

            if mode == "fp32":
                # normalize x in place: x *= (1/norm) broadcast over C.
                # 1/norm row is partition-broadcast by DMA (step-0 AP).
                for cc in range(NCHUNK):
                    sl = ts(cc, CHUNK)
                    rnb = rpool.tile([128, CHUNK], f32)
                    nc.sync.dma_start(
                        rnb, rn_dram[None, ts(cc, CHUNK)].to_broadcast([128, CHUNK])
                    )
                    nc.vector.tensor_mul(xA[:, sl], xA[:, sl], rnb)
                    nc.gpsimd.tensor_mul(xB[0:64, sl], xB[0:64, sl], rnb[0:64, :])

                for t in range(nt):
                    tsl = ts(t, 128)
                    g = gpool.tile([128, N], f32)
                    for cc in range(NCHUNK):
                        sl = ts(cc, CHUNK)
                        ps = gpsum.tile([128, CHUNK], f32)
                        nc.tensor.matmul(
                            ps, xA[:, tsl], xA[:, sl], start=True, stop=False
                        )
                        nc.tensor.matmul(
                            ps, xB[:, tsl], xB[:, sl], start=False, stop=True
                        )
                        nc.scalar.copy(g[:, sl], ps)
                    # knock out self-match diagonal (query p == column 128t+p)
                    nc.vector.tensor_add(g[:, tsl], g[:, tsl], eye)
                    v8 = vpool.tile([128, 8], f32)
                    nc.vector.max(out=v8, in_=g)
                    i8 = vpool.tile([128, 8], u16)
                    nc.vector.max_index(i8, v8, g)
                    nc.sync.dma_start(idx_out[tsl, :], i8)
            else:
                # fp16 split of the normalized points: xn = h + l/32 + O(2^-24)
                #   h  = fp16(xn)          l5 = fp16((xn - h) * 32)
                #   h5 = fp16(h / 32)
                # Gram accumulates h.h + h.(l/32*32) terms with exactly
                # cancelling power-of-two scales:
                #   h[t] x h  +  h5[t] x l5  +  l5[t] x h5
                hA = xpool.tile([128, N], f16)
                hB = xpool.tile([128, N], f16)
                h5A = xpool.tile([128, N], f16)
                h5B = xpool.tile([128, N], f16)
                l5A = xpool.tile([128, N], f16)
                l5B = xpool.tile([128, N], f16)

                for cc in range(NCHUNK):
                    sl = ts(cc, CHUNK)
                    xa = spool.tile([128, CHUNK], f32, tag="xa")
                    nc.sync.dma_start(xa, xin[0:128, sl])
                    xb = spool.tile([128, CHUNK], f32, tag="xb")
                    nc.gpsimd.memset(xb[64:128, :], 0.0)
                    nc.sync.dma_start(xb[0:64, :], xin[128:192, sl])
                    rnb = rpool.tile([128, CHUNK], f32)
                    nc.sync.dma_start(
                        rnb, rn_dram[None, ts(cc, CHUNK)].to_broadcast([128, CHUNK])
                    )
                    nc.vector.tensor_mul(xa, xa, rnb)  # xa = xn (A half)
                    nc.vector.tensor_mul(xb, xb, rnb)
                    nc.scalar.copy(hA[:, sl], xa)  # cast to fp16
                    nc.scalar.copy(hB[:, sl], xb)
                    nc.vector.tensor_sub(xa, xa, hA[:, sl])  # xa = xn - h
                    nc.vector.tensor_sub(xb, xb, hB[:, sl])
                    nc.scalar.mul(l5A[:, sl], xa, 32.0)
                    nc.scalar.mul(l5B[:, sl], xb, 32.0)
                    nc.scalar.mul(h5A[:, sl], hA[:, sl], 0.03125)
                    nc.scalar.mul(h5B[:, sl], hB[:, sl], 0.03125)

                for t in range(nt):
                    tsl = ts(t, 128)
                    g = gpool.tile([128, N], f32)
                    for cc in range(NCHUNK):
                        sl = ts(cc, CHUNK)
                        ps = gpsum.tile([128, CHUNK], f32)
                        nc.tensor.matmul(
                            ps, hA[:, tsl], hA[:, sl], start=True, stop=False
                        )
                        nc.tensor.matmul(
                            ps, hB[:, tsl], hB[:, sl], start=False, stop=False
                        )
                        nc.tensor.matmul(
                            ps, h5A[:, tsl], l5A[:, sl], start=False, stop=False
                        )
                        nc.tensor.matmul(
                            ps, h5B[:, tsl], l5B[:, sl], start=False, stop=False
                        )
                        nc.tensor.matmul(
                            ps, l5A[:, tsl], h5A[:, sl], start=False, stop=False
                        )
                        nc.tensor.matmul(
                            ps, l5B[:, tsl], h5B[:, sl], start=False, stop=True
                        )
                        nc.scalar.copy(g[:, sl], ps)
                    nc.vector.tensor_add(g[:, tsl], g[:, tsl], eye)
                    v8 = vpool.tile([128, 8], f32)
                    nc.vector.max(out=v8, in_=g)
                    i8 = vpool.tile([128, 8], u16)
                    nc.vector.max_index(i8, v8, g)
                    nc.sync.dma_start(idx_out[tsl, :], i8)

    nc.compile()
    return nc


def _get_nc():
    if "nc" not in _cache:
        _cache["nc"] = _build_nc()
    return _cache["nc"]


def shard_inputs(x):
    """x: [B, C, N, 1] -> list of 8 per-core input maps (rotated columns)."""
    xs = np.ascontiguousarray(np.asarray(x, dtype=np.float32).reshape(B, C, N))
    in_maps = []
    for c in range(NCORES):
        b, r = divmod(c, 4)
        s = r * RBLK
        xb = xs[b]
        rot = np.ascontiguousarray(np.roll(xb, -s, axis=1)) if s else xb
        in_maps.append({"xin": rot})
    return in_maps


def assemble(results):
    """results: list of 8 dicts with 'idx8' [RBLK, 8] uint16 -> edge_index."""
    nn = np.empty((B, N, 9), np.int32)
    for c in range(NCORES):
        b, r = divmod(c, 4)
        s = r * RBLK
        i8 = results[c]["idx8"].astype(np.int64)
        nn[b, s : s + RBLK, 1:9] = (i8 + s) % N
        nn[b, s : s + RBLK, 0] = np.arange(s, s + RBLK)
    center = np.broadcast_to(np.arange(N, dtype=np.int32)[None, :, None], (B, N, 9))
    return np.ascontiguousarray(np.stack([nn, center], axis=0).astype(np.int32))


def kernel(x, _trace=False, **trace_kwargs):
    from concourse.bass_utils import run_bass_kernel_spmd

    nc = _get_nc()
    in_maps = shard_inputs(x)
    res = run_bass_kernel_spmd(
        nc, in_maps, core_ids=list(range(NCORES)), trace=_trace, **trace_kwargs
    )
    _cache["last_results"] = res
    return assemble(res.results)
